# revision 13
# baseline (speedup 1.0000x reference)
"""GQA causal attention layer (QKV proj + NeoX RoPE + softmax attention + o_proj)
for Trainium2, tensor-parallel over heads across 8 NeuronCores.

Problem shapes (hardcoded): B=1, S=2048, HID=2048, NH=32, NKV=8, HD=64.
Per core c: 4 query heads (4c..4c+3) + 1 kv head (c).

v2 design notes (vs v1 baseline at ~175us):
  - bf16 on every matmul operand (accumulation stays fp32 in PSUM); fp16 y
    out. Halves HBM traffic and DVE elementwise cost (2x bf16 perf mode).
  - Single interleaved emission stream keeps the PE busy end-to-end so the
    HAM clock gate stays at 8/8 (2.4 GHz): attention j=0 is interleaved with
    the m-chunk-1 QKV projection, o_proj half-0 is interleaved into
    attention j=1, o_proj half-1 is the tail.
  - Attention pipelined at 512-col granularity: st pool bufs=4 (4 PSUM
    banks) + pv [128,1024] (2 banks) + 2 banks for interleaved qkv/o_proj.
  - Scalar engine does ONLY the exps (the phase-2 co-bottleneck);
    masks/copies/normalize live on gpsimd(pool)/vector.
  - Row-sums via ones-columns in the PV stationary; vaug_e=[v|1] for even
    heads, vaug_o=[1|v] for odd heads so normalize is partition-aligned
    for both (attn at rows 0:64 / 64:128 resp).
"""

import numpy as np

import concourse.bass as bass
import concourse.mybir as mybir
import concourse.tile as tile
from concourse import bacc
from concourse import bass_utils
from concourse.masks import make_identity

B, S, HID = 1, 2048, 2048
NH, NKV, HD = 32, 8, 64
NCORES = 8
HPC = NH // NCORES          # 4 query heads per core
ROPE_BASE = 10000.0
SCALE = 1.0 / np.sqrt(HD)   # 0.125
NEG = -1e9

F32 = mybir.dt.float32
BF16 = mybir.dt.bfloat16
F16 = mybir.dt.float16

KT = S // 128               # 16 k-tiles of 128
MC = 1024                   # phase-1 m-chunk
QCHUNK = 1024               # attention q-chunk
HORDER = (1, 0, 3, 2)       # odd heads first (no kdup dependency)


def build_kernel(passes=1, debug_dump=False):
    nc = bacc.Bacc("TRN2", target_bir_lowering=False, debug=False,
                   num_devices=NCORES)

    xT3 = nc.dram_tensor("xT3", [128, KT, S], BF16, kind="ExternalInput").ap()
    w3 = nc.dram_tensor("w3", [128, KT, 384], BF16, kind="ExternalInput").ap()
    wo = nc.dram_tensor("wo", [256, HID], BF16, kind="ExternalInput").ap()
    Cr = nc.dram_tensor("Cr", [128, S], F32, kind="ExternalInput").ap()
    Sr = nc.dram_tensor("Sr", [128, S], F32, kind="ExternalInput").ap()
    trimask = nc.dram_tensor("trimask", [128, 128], BF16,
                             kind="ExternalInput").ap()
    yT3 = nc.dram_tensor("yT3", [128, KT, S], F16, kind="ExternalOutput").ap()
    dbg = {}
    if debug_dump:
        for nm, shp, dt in [("dqr0", [128, S], BF16), ("dqr1", [128, S], BF16),
                            ("dkr", [128, S], BF16), ("dve", [128, S], BF16),
                            ("dvo", [128, S], BF16), ("dos0", [128, S], BF16),
                            ("dos1", [128, S], BF16), ("dqc", [128, MC], BF16),
                            ("dpt0", [128, 512], BF16), ("dpvs", [128, MC], F32),
                            ("dsums", [128, MC], F32), ("drec", [128, MC], F32),
                            ("dxb", [128, 2048], BF16), ("dwsb", [128, KT * 384], BF16)]:
            dbg[nm] = nc.dram_tensor(nm, shp, dt, kind="ExternalOutput").ap()

    with tile.TileContext(nc) as tc:
      for _pass in range(passes):
        with (
            tc.tile_pool(name="persist", bufs=1, side=None) as pers,
            tc.tile_pool(name="xp1", bufs=1) as xp1,
        ):
            # ---- persistent tiles ----
            qr = [pers.tile([128, S], BF16, tag=f"qr{t}", name=f"qr{t}")
                  for t in range(2)]
            kr = pers.tile([128, S], BF16, tag="kr")
            outstat = [pers.tile([128, S], BF16, tag=f"os{p}", name=f"os{p}")
                       for p in range(2)]
            wo_sb = [pers.tile([128, HID], BF16, tag=f"wo{p}", name=f"wo{p}")
                     for p in range(2)]
            Ct = pers.tile([128, S], F32, tag="Ct")
            St = pers.tile([128, S], F32, tag="St")
            wsb = pers.tile([128, KT * 384], BF16, tag="wsb")
            vaug_e = pers.tile([128, S], BF16, tag="vaug_e")
            vaug_o = pers.tile([128, S], BF16, tag="vaug_o")
            trim = pers.tile([128, 128], BF16, tag="trim")
            ident = pers.tile([128, 128], BF16, tag="ident")

            xp0 = tc.alloc_tile_pool(name="xp0", bufs=1)
            xb0 = [xp0.tile([128, 2048], BF16, tag=f"xb0_{b}",
                            name=f"xb0_{b}") for b in range(8)]
            xb1 = [xp1.tile([128, 2048], BF16, tag=f"xb1_{b}",
                            name=f"xb1_{b}") for b in range(8)]

            # ---- preload DMAs ----
            nc.gpsimd.dma_start(wsb, w3)
            for b in range(8):
                nc.scalar.dma_start(xb0[b], xT3[:, 2 * b:2 * b + 2, 0:MC])
            for b in range(8):
                nc.sync.dma_start(xb1[b], xT3[:, 2 * b:2 * b + 2, MC:S])
            nc.gpsimd.dma_start(Ct, Cr)
            nc.gpsimd.dma_start(St, Sr)
            nc.gpsimd.dma_start(trim, trimask)
            for p in range(2):
                nc.gpsimd.dma_start(wo_sb[p], wo[128 * p:128 * (p + 1), :])
            for i in range(KT):
                nc.gpsimd.memset(vaug_e[:, 128 * i + 64:128 * (i + 1)], 1.0)
                nc.gpsimd.memset(vaug_o[:, 128 * i:128 * i + 64], 1.0)
            make_identity(nc, ident)

            # ====== helpers ======
            def _copy(eng, out, in_):
                if eng is nc.scalar:
                    nc.scalar.copy(out, in_)
                else:
                    eng.tensor_copy(out, in_)

            def qkv_group(ps, n, c, xsrc):
                """16 accumulating matmuls for qkv output tile n, chunk c."""
                for k in range(KT):
                    nc.tensor.matmul(
                        ps[:, 0:512],
                        wsb[:, 384 * k + 128 * n:384 * k + 128 * (n + 1)],
                        xsrc[k // 2][:, (k % 2) * MC + 512 * c:
                                     (k % 2) * MC + 512 * (c + 1)],
                        start=(k == 0), stop=(k == KT - 1))

            def evict_group(ps, n, c, m0, qC, qS, vsb):
                """TT-evict psum group (n,c) through the RoPE C/S muls."""
                a0 = 512 * c          # col offset within the m-chunk
                g0 = m0 + a0          # absolute col offset
                if n < 2:
                    nc.vector.tensor_mul(qC[n][:, a0:a0 + 512], ps[:, 0:512],
                                         Ct[:, g0:g0 + 512])
                    nc.vector.tensor_mul(qS[n][:, a0:a0 + 512], ps[:, 0:512],
                                         St[:, g0:g0 + 512])
                else:
                    nc.vector.tensor_mul(qC[2][64:128, a0:a0 + 512],
                                         ps[64:128, 0:512],
                                         Ct[64:128, g0:g0 + 512])
                    nc.vector.tensor_mul(qS[2][64:128, a0:a0 + 512],
                                         ps[64:128, 0:512],
                                         St[64:128, g0:g0 + 512])
                    nc.vector.tensor_copy(vsb[:, a0:a0 + 512],
                                          ps[0:64, 0:512])

            def rope_add(m0, qC, qS, swp):
                """Swap-halves DMAs + final adds into qr/kr for one m-chunk."""
                for t in range(3):
                    r0, r1 = (0, 128) if t < 2 else (64, 128)
                    sw = swp.tile([128, MC], BF16, tag=f"sw{t}",
                                  name=f"sw{t}_{m0}")
                    for g in range(r0 // 32, r1 // 32, 2):
                        nc.gpsimd.dma_start(
                            sw[32 * g:32 * g + 32, :],
                            qS[t][32 * g + 32:32 * g + 64, :])
                        nc.gpsimd.dma_start(
                            sw[32 * g + 32:32 * g + 64, :],
                            qS[t][32 * g:32 * g + 32, :])
                    dst = qr[t] if t < 2 else kr
                    nc.vector.tensor_add(dst[r0:r1, m0:m0 + MC],
                                         qC[t][r0:r1, :], sw[r0:r1, :])
                nc.gpsimd.dma_start(kr[0:64, m0:m0 + MC],
                                    kr[64:128, m0:m0 + MC])

            def transposes(mc, vsb, trp):
                for i in range(8 * mc, 8 * (mc + 1)):
                    a0 = 128 * (i - 8 * mc)
                    tp = trp.tile([128, 64], BF16, tag="tp", name=f"tp{i}")
                    nc.tensor.transpose(tp, vsb[:, a0:a0 + 128],
                                        ident[0:64, 0:64])
                    nc.vector.tensor_copy(vaug_e[:, 128 * i:128 * i + 64], tp)
                    nc.vector.tensor_copy(
                        vaug_o[:, 128 * i + 64:128 * (i + 1)], tp)

            # ====== MC0: QKV proj for m in [0, 1024) ======
            # (pool release must be LIFO per memory space)
            qkv6 = tc.alloc_tile_pool(name="qkv6", bufs=1, space="PSUM")
            trp0 = tc.alloc_tile_pool(name="trp0", bufs=2, space="PSUM")
            ev0 = tc.alloc_tile_pool(name="ev0", bufs=1)
            swp0 = tc.alloc_tile_pool(name="swp0", bufs=1)

            ps0 = [qkv6.tile([128, 512], F32, tag=f"mc0ps{n}_{c}",
                             name=f"mc0ps{n}_{c}")
                   for n in range(3) for c in range(2)]
            for b in range(8):
                for k in (2 * b, 2 * b + 1):
                    for n in range(3):
                        for c in range(2):
                            nc.tensor.matmul(
                                ps0[2 * n + c][:, 0:512],
                                wsb[:, 384 * k + 128 * n:
                                    384 * k + 128 * (n + 1)],
                                xb0[b][:, (k % 2) * MC + 512 * c:
                                       (k % 2) * MC + 512 * (c + 1)],
                                start=(k == 0), stop=(k == KT - 1))
            qC0 = [ev0.tile([128, MC], BF16, tag=f"qC0_{t}",
                            name=f"qC0_{t}") for t in range(3)]
            qS0 = [ev0.tile([128, MC], BF16, tag=f"qS0_{t}",
                            name=f"qS0_{t}") for t in range(3)]
            vsb0 = ev0.tile([64, MC], BF16, tag="vsb0")
            for n in range(3):
                for c in range(2):
                    evict_group(ps0[2 * n + c], n, c, 0, qC0, qS0, vsb0)
            rope_add(0, qC0, qS0, swp0)
            transposes(0, vsb0, trp0)
            if debug_dump:
                nc.sync.dma_start(dbg["dqc"], qC0[0])

            swp0.release()
            ev0.release()
            trp0.release()
            qkv6.release()
            xp0.release()

            # ====== attention pools (live j0 through oproj) ======
            stp = tc.alloc_tile_pool(name="stp", bufs=4, space="PSUM")
            pvp = tc.alloc_tile_pool(name="pvp", bufs=1, space="PSUM")
            ptp = tc.alloc_tile_pool(name="ptp", bufs=6)
            nrm = tc.alloc_tile_pool(name="nrm", bufs=2)

            def emit_attn(j, h, absorber=None):
                half, p = h % 2, h // 2
                qrow = 64 * half
                kb = 64 * half
                va = vaug_o if half else vaug_e
                ilast = 8 * (j + 1) - 1
                pv = pvp.tile([128, QCHUNK], F32, tag="pv", name=f"pv{j}_{h}")
                ab = 0
                for i in range(8 * (j + 1)):
                    qstart = max(QCHUNK * j, 128 * i)
                    qlen = QCHUNK * (j + 1) - qstart
                    nch = (qlen + 511) // 512
                    for c in range(nch):
                        cols = min(512, qlen - 512 * c)
                        st = stp.tile([128, 512], F32, tag="st",
                                      name=f"st{j}_{h}_{i}_{c}")
                        nc.tensor.matmul(
                            st[:, 0:cols],
                            kr[kb:kb + 64, 128 * i:128 * (i + 1)],
                            qr[p][qrow:qrow + 64,
                                  qstart + 512 * c:qstart + 512 * c + cols],
                            start=True, stop=True)
                        pt = ptp.tile([128, 512], BF16, tag="pt",
                                      name=f"pt{j}_{h}_{i}_{c}")
                        nc.scalar.activation(
                            pt[:, 0:cols], st[:, 0:cols],
                            mybir.ActivationFunctionType.Exp, scale=SCALE)
                        if c == 0 and 128 * i >= QCHUNK * j:
                            nc.gpsimd.tensor_mul(pt[:, 0:128],
                                                 pt[:, 0:128], trim)
                        if dbg and (j, h, i, c) == (0, 1, 0, 0):
                            nc.sync.dma_start(dbg["dpt0"], pt)
                        off = qstart - QCHUNK * j + 512 * c
                        nc.tensor.matmul(
                            pv[:, off:off + cols],
                            va[:, 128 * i:128 * (i + 1)], pt[:, 0:cols],
                            start=(i == 0), stop=(i == ilast))
                    if absorber is not None and i % 2 == 1:
                        absorber(ab)
                        ab += 1
                # ---- evict + normalize (off critical path) ----
                pvs = nrm.tile([128, QCHUNK], F32, tag="pvs",
                               name=f"pvs{j}_{h}")
                nc.vector.tensor_copy(pvs, pv)
                rec = nrm.tile([128, QCHUNK], F32, tag="rec",
                               name=f"rec{j}_{h}")
                # reciprocal_approx_fast only works based at partition 0
                if half == 0:
                    sums = nrm.tile([128, QCHUNK], F32, tag="sums",
                                    name=f"sums{j}_{h}")
                    nc.gpsimd.dma_start(sums[0:64, :], pvs[64:128, :])
                    nc.vector.reciprocal_approx_fast(rec[0:64, :],
                                                     sums[0:64, :])
                    nc.vector.tensor_mul(
                        outstat[p][0:64, QCHUNK * j:QCHUNK * (j + 1)],
                        pvs[0:64, :], rec[0:64, :])
                else:
                    nc.vector.reciprocal_approx_fast(rec[0:64, :],
                                                     pvs[0:64, :])
                    nc.gpsimd.dma_start(rec[64:128, :], rec[0:64, :])
                    nc.vector.tensor_mul(
                        outstat[p][64:128, QCHUNK * j:QCHUNK * (j + 1)],
                        pvs[64:128, :], rec[64:128, :])
                if dbg and (j, h) == (0, 1):
                    nc.sync.dma_start(dbg["dpvs"], pvs)
                    nc.sync.dma_start(dbg["dsums"], rec)
                    nc.sync.dma_start(dbg["drec"], rec)

            # ====== J0 interleaved with MC1 QKV ======
            ev1 = tc.alloc_tile_pool(name="ev1", bufs=1)
            swp1 = tc.alloc_tile_pool(name="swp1", bufs=1)
            qC1 = [ev1.tile([128, MC], BF16, tag=f"qC1_{t}",
                            name=f"qC1_{t}") for t in range(3)]
            qS1 = [ev1.tile([128, MC], BF16, tag=f"qS1_{t}",
                            name=f"qS1_{t}") for t in range(3)]
            vsb1 = ev1.tile([64, MC], BF16, tag="vsb1")

            qkv2 = tc.alloc_tile_pool(name="qkv2", bufs=2, space="PSUM")
            groups = [(n, c) for n in range(3) for c in range(2)]
            for hi, h in enumerate(HORDER[:3]):
                emit_attn(0, h)
                for (n, c) in groups[2 * hi:2 * hi + 2]:
                    psq = qkv2.tile([128, 512], F32, tag="psq",
                                    name=f"psq{n}_{c}")
                    qkv_group(psq, n, c, xb1)
                    evict_group(psq, n, c, MC, qC1, qS1, vsb1)
            rope_add(MC, qC1, qS1, swp1)
            qkv2.release()

            trp1 = tc.alloc_tile_pool(name="trp1", bufs=2, space="PSUM")
            emit_attn(0, HORDER[3])
            transposes(1, vsb1, trp1)
            trp1.release()
            swp1.release()
            ev1.release()

            # ====== J1 with o_proj half-0 interleaved ======
            oprp = tc.alloc_tile_pool(name="oprp", bufs=2, space="PSUM")
            ysbp = tc.alloc_tile_pool(name="ysbp", bufs=3)
            ysb_jobs = {}

            def oproj_job(nt, c, mcol, cp_engines):
                """One [128,512] o_proj chunk: 2 matmuls + fp16 copy + store."""
                pso = oprp.tile([128, 512], F32, tag="pso",
                                name=f"pso{nt}_{c}_{mcol}")
                for p in range(2):
                    nc.tensor.matmul(
                        pso[:, 0:512],
                        wo_sb[p][:, 128 * nt:128 * (nt + 1)],
                        outstat[p][:, mcol + 512 * c:mcol + 512 * (c + 1)],
                        start=(p == 0), stop=(p == 1))
                pair, slot = nt // 2, nt % 2
                ysbt = ysb_jobs.get((pair, mcol))
                if ysbt is None:
                    ysbt = ysbp.tile([128, 2048], F16, tag="ysb",
                                     name=f"ysb{pair}_{mcol}")
                    ysb_jobs[(pair, mcol)] = ysbt
                eng = cp_engines[(nt + c) % len(cp_engines)]
                _copy(eng, ysbt[:, slot * MC + 512 * c:
                                slot * MC + 512 * (c + 1)], pso[:, 0:512])
                if slot == 1 and c == 1:
                    nc.sync.dma_start(
                        yT3[:, 2 * pair:2 * pair + 2, mcol:mcol + MC], ysbt)
                    del ysb_jobs[(pair, mcol)]

            for hi, h in enumerate(HORDER):
                jobs = [(nt, c)
                        for nt in range(4 * hi, 4 * hi + 4) for c in range(2)]

                def absorber(ab, jobs=jobs):
                    nt, c = jobs[ab]
                    oproj_job(nt, c, 0, (nc.vector,))

                emit_attn(1, h, absorber=absorber)

            # ====== o_proj half-1 tail ======
            for nt in range(KT):
                for c in range(2):
                    oproj_job(nt, c, MC, (nc.scalar, nc.vector))

            if debug_dump:
                nc.sync.dma_start(dbg["dqr0"], qr[0])
                nc.sync.dma_start(dbg["dqr1"], qr[1])
                nc.sync.dma_start(dbg["dkr"], kr)
                nc.sync.dma_start(dbg["dve"], vaug_e)
                nc.sync.dma_start(dbg["dvo"], vaug_o)
                nc.sync.dma_start(dbg["dos0"], outstat[0])
                nc.sync.dma_start(dbg["dos1"], outstat[1])
                nc.sync.dma_start(dbg["dxb"], xb1[0])
                nc.sync.dma_start(dbg["dwsb"], wsb)

            ysbp.release()
            oprp.release()
            nrm.release()
            ptp.release()
            pvp.release()
            stp.release()

    nc.compile()
    return nc


def make_host_inputs(x, w_qkv, w_o):
    """Host-side prep: tiled/transposed bf16 inputs, rope tables."""
    import ml_dtypes
    bf16 = ml_dtypes.bfloat16
    x = np.asarray(x, dtype=np.float32)
    w_qkv = np.asarray(w_qkv, dtype=np.float32)
    w_o = np.asarray(w_o, dtype=np.float32)
    xT = np.ascontiguousarray(x.reshape(S, HID).T)          # [HID, S]
    xT3 = np.ascontiguousarray(
        xT.reshape(KT, 128, S).transpose(1, 0, 2)).astype(bf16)

    inv_freq = 1.0 / (ROPE_BASE ** (np.arange(0, HD, 2, dtype=np.float32) / HD))
    t = np.arange(S, dtype=np.float32)
    freqs = np.outer(t, inv_freq)                     # [S, 32]
    cosT = np.cos(freqs).T.astype(np.float32)         # [32, S]
    sinT = np.sin(freqs).T.astype(np.float32)
    C = np.ascontiguousarray(np.tile(cosT, (4, 1)))   # [128, S]
    # v2 applies S BEFORE the row swap, so the sign pattern rides along with
    # the swap: rows 0:32 = +sin, 32:64 = -sin (swapped vs the classic table).
    Sn = np.ascontiguousarray(np.tile(np.concatenate([sinT, -sinT], 0),
                                      (2, 1)))

    r = np.arange(128)
    trimask = np.where(r[None, :] >= r[:, None], np.float32(1.0),
                       np.float32(0.0)).astype(bf16)

    in_maps = []
    for c in range(NCORES):
        qcols = np.arange(4 * c * HD, 4 * (c + 1) * HD)
        vcols = NH * HD + NKV * HD + np.arange(c * HD, (c + 1) * HD)
        kcols = NH * HD + np.arange(c * HD, (c + 1) * HD)
        w_stat = np.concatenate(
            [w_qkv[:, qcols], w_qkv[:, vcols], w_qkv[:, kcols]], axis=1)
        w3c = np.ascontiguousarray(
            w_stat.reshape(KT, 128, 384).transpose(1, 0, 2)).astype(bf16)
        wo_c = np.ascontiguousarray(
            w_o[256 * c:256 * (c + 1), :]).astype(bf16)
        in_maps.append({
            "xT3": xT3, "w3": w3c, "wo": wo_c,
            "Cr": C, "Sr": Sn, "trimask": trimask,
        })
    return in_maps


_NC_CACHE = {}


def get_nc():
    if "nc" not in _NC_CACHE:
        _NC_CACHE["nc"] = build_kernel()
    return _NC_CACHE["nc"]


def _get_exec():
    """Build (once) the jitted sharded executable over the 8 cores."""
    if "exec" in _NC_CACHE:
        return _NC_CACHE["exec"]
    import jax
    from jax.sharding import Mesh, PartitionSpec, NamedSharding
    from jax.experimental.shard_map import shard_map
    from concourse import bass2jax

    nc = get_nc()
    bass2jax.install_neuronx_cc_hook()
    partition_name = (nc.partition_id_tensor.name
                      if nc.partition_id_tensor else None)
    in_names, out_names, out_avals, zero_outs = [], [], [], []
    for alloc in nc.m.functions[0].allocations:
        if not isinstance(alloc, mybir.MemoryLocationSet):
            continue
        name = alloc.memorylocations[0].name
        if alloc.kind == "ExternalInput":
            if name != partition_name:
                in_names.append(name)
        elif alloc.kind == "ExternalOutput":
            shape = tuple(alloc.tensor_shape)
            dtype = mybir.dt.np(alloc.dtype)
            out_names.append(name)
            out_avals.append(jax.core.ShapedArray(shape, dtype))
            zero_outs.append(np.zeros(shape, dtype))
    n_params = len(in_names)
    all_in = list(in_names) + list(out_names)
    if partition_name is not None:
        all_in.append(partition_name)

    def _body(*args):
        operands = list(args)
        if partition_name is not None:
            operands.append(bass2jax.partition_id_tensor())
        return tuple(bass2jax._bass_exec_p.bind(
            *operands, out_avals=tuple(out_avals), in_names=tuple(all_in),
            out_names=tuple(out_names), lowering_input_output_aliases=(),
            sim_require_finite=True, sim_require_nnan=True, nc=nc))

    devices = jax.devices()[:NCORES]
    mesh = Mesh(np.asarray(devices), ("core",))
    REPL = {"xT3", "Cr", "Sr", "trimask"}
    in_specs = tuple(PartitionSpec() if n in REPL else PartitionSpec("core")
                     for n in in_names)
    in_specs = in_specs + (PartitionSpec("core"),) * len(out_names)
    f = jax.jit(shard_map(_body, mesh=mesh, in_specs=in_specs,
                          out_specs=(PartitionSpec("core"),) * len(out_names),
                          check_rep=False), keep_unused=True)
    sh = NamedSharding(mesh, PartitionSpec("core"))
    shr = NamedSharding(mesh, PartitionSpec())
    _NC_CACHE["exec"] = (f, in_names, out_names, zero_outs, sh, shr, REPL)
    return _NC_CACHE["exec"]


def kernel(x, w_qkv, w_o):
    import jax

    f, in_names, out_names, zero_outs, sh, shr, REPL = _get_exec()
    in_maps = make_host_inputs(x, w_qkv, w_o)
    args = []
    for name in in_names:
        if name in REPL:
            args.append(jax.device_put(in_maps[0][name], shr))
        else:
            args.append(jax.device_put(
                np.concatenate([m[name] for m in in_maps], 0), sh))
    if "zeros" not in _NC_CACHE:
        _NC_CACHE["zeros"] = [
            jax.device_put(
                np.zeros((NCORES * z.shape[0], *z.shape[1:]), z.dtype), sh)
            for z in zero_outs]
    args += _NC_CACHE["zeros"]
    outs = f(*args)
    y_idx = out_names.index("yT3")
    if "reduce" not in _NC_CACHE:
        import jax.numpy as jnp

        def _reduce(a):
            # a: [8*128, KT, S] fp16 partials -> [S, HID] f32
            a = jnp.sum(jnp.reshape(a, (NCORES, 128, KT, S))
                        .astype(jnp.float32), axis=0)      # [128, KT, S]
            a = jnp.transpose(a, (1, 0, 2))                # [KT, 128, S]
            return jnp.transpose(jnp.reshape(a, (HID, S)))  # [S, HID]
        _NC_CACHE["reduce"] = jax.jit(_reduce)
    out = np.asarray(_NC_CACHE["reduce"](outs[y_idx]))
    return np.ascontiguousarray(out.astype(np.float32)).reshape(B, S, HID)
